# revision 1
# baseline (speedup 1.0000x reference)
"""Trainium2 Bass kernel for nn_AnswerDecoder (LSTM decoder + vocab projection).

Sharding: vocab-parallel across 8 NeuronCores (each core owns V/8 = 2500 rows
of W_vocab and produces logits[:, :, k*2500:(k+1)*2500]); the LSTM itself is
replicated on every core (its cost is latency-bound, not FLOP-bound, so
replication is free compared to the per-step all-gather a hidden-shard would
need). No collectives.

Numerics: all matmuls bf16 with fp32 PSUM accumulation; LSTM cell state c is
kept fp32; h is rounded to bf16 each step (validated: end-to-end rel err
~3e-3 vs fp32 reference). Logits leave the device as bf16 and are cast to
fp32 on the host.

Layout: gate columns are host-permuted so each step's gates land in two
[128, 512] PSUM banks via column-tiled (2x64) matmuls: partitions =
(hidden-half, batch), bank0 free dim = [i|f], bank1 = [g|o], each gate slice
holding units of that half. Biases are folded in as a K=1 matmul against a
ones row. h^T for the next step's stationary operand is produced with 4
identity matmuls (PE transpose) into PSUM and copied into a persistent
H^T buffer that also feeds the vocab matmuls (which run with a one-pair lag
so they fill the PE idle gaps left by the activation chain).
"""
import os
import sys
import types

import numpy as np

import concourse.bass as bass
import concourse.bacc as bacc
import concourse.mybir as mybir
from concourse import tile
from concourse.bass_utils import run_bass_kernel_spmd

dt = mybir.dt
AF = mybir.ActivationFunctionType

B, T = 64, 64
Q, E, H, V = 512, 256, 512, 20000
NCORES = 8
VS = V // NCORES          # 2500 vocab rows per core
TB = T * B                # 4096 tokens
NVT = 5                   # vocab N-tiles per 128-token chunk
VT = VS // NVT            # 500
START_IDX = 1


def _gate_perm():
    """new gate-column index -> original gate-column index.

    bank0 = [i|g] (both inputs of the early i*g product), bank1 = [f|o]
    (consumed late in the chain), so the c-update critical path starts as
    soon as bank1's matmuls land."""
    gate_of = {0: (0, 2), 1: (1, 3)}   # bank -> (q for j<256, q for j>=256)
    perm = np.empty(4 * H, dtype=np.int64)
    for bank in range(2):
        for hh in range(2):
            for j in range(512):
                q = gate_of[bank][1 if j >= 256 else 0]
                u = 256 * hh + (j % 256)
                perm[bank * 1024 + hh * 512 + j] = q * H + u
    return perm


def build(nc):
    f32, bf16 = dt.float32, dt.bfloat16

    xst_d = nc.declare_dram_parameter("xst", [E, TB], bf16, isOutput=False)
    wcat_d = nc.declare_dram_parameter("wcat", [H + E + 1, 4 * H], bf16, isOutput=False)
    wvt_d = nc.declare_dram_parameter("wvt", [H + 1, VS], bf16, isOutput=False)
    qvt_d = nc.declare_dram_parameter("qvt", [Q, B], bf16, isOutput=False)
    wht_d = nc.declare_dram_parameter("wht", [Q, H], bf16, isOutput=False)
    wct_d = nc.declare_dram_parameter("wct", [Q, H], bf16, isOutput=False)
    ident_d = nc.declare_dram_parameter("ident", [128, 128], bf16, isOutput=False)
    ones_d = nc.declare_dram_parameter("ones", [1, 128], bf16, isOutput=False)
    out_d = nc.declare_dram_parameter("out", [TB, VS], bf16, isOutput=True)

    with tile.TileContext(nc) as tc:
        with (
            tc.tile_pool(name="const", bufs=1) as const,
            tc.tile_pool(name="work", bufs=2) as work,
            tc.tile_pool(name="hbf", bufs=2) as hpool,
            tc.tile_pool(name="stage", bufs=2) as stpool,
            tc.tile_pool(name="pgate", bufs=2, space="PSUM") as pg,
            tc.tile_pool(name="ptrans", bufs=1, space="PSUM") as pt,
            tc.tile_pool(name="pvocab", bufs=NVT, space="PSUM") as pv,
        ):
            # ---- load constants -------------------------------------------------
            xst = const.tile([128, 2 * TB], bf16)           # [128, (xc, token)]
            nc.sync.dma_start(
                xst[:].rearrange("p (c n) -> p c n", c=2),
                xst_d[:].rearrange("(c p) n -> p c n", p=128),
            )
            wcat = const.tile([128, 6 * 4 * H], bf16)       # [128, (kc, gatecol)]
            nc.sync.dma_start(
                wcat[:].rearrange("p (c n) -> p c n", c=6),
                wcat_d[0 : H + E, :].rearrange("(c p) n -> p c n", p=128),
            )
            wbias = const.tile([1, 4 * H], bf16)
            nc.sync.dma_start(wbias[:], wcat_d[H + E : H + E + 1, :])

            qvt = const.tile([128, 4 * B], bf16)            # [128, (kc, b)]
            nc.sync.dma_start(
                qvt[:].rearrange("p (c n) -> p c n", c=4),
                qvt_d[:].rearrange("(c p) n -> p c n", p=128),
            )
            wht = const.tile([128, 4 * H], bf16)            # [128, (kc, unit)]
            nc.sync.dma_start(
                wht[:].rearrange("p (c n) -> p c n", c=4),
                wht_d[:].rearrange("(c p) n -> p c n", p=128),
            )
            wct = const.tile([128, 4 * H], bf16)
            nc.sync.dma_start(
                wct[:].rearrange("p (c n) -> p c n", c=4),
                wct_d[:].rearrange("(c p) n -> p c n", p=128),
            )
            ident = const.tile([128, 128], bf16)
            nc.sync.dma_start(ident[:], ident_d[:])
            ones = const.tile([1, 128], bf16)
            nc.sync.dma_start(ones[:], ones_d[:])

            wvt = const.tile([128, 4 * VS], bf16)           # [128, (kc, vocab)]
            nc.sync.dma_start(
                wvt[:].rearrange("p (c n) -> p c n", c=4),
                wvt_d[0:H, :].rearrange("(c p) n -> p c n", p=128),
            )
            # b_vocab replicated across all 128 partitions (partition step 0)
            bvoc = const.tile([128, VS], bf16)
            nc.sync.dma_start(
                bvoc[:], wvt_d[H : H + 1, :].broadcast_to([128, VS])
            )

            H_allT = const.tile([128, 4 * TB], bf16)        # [128, (kc, token)]

            # ---- h0^T = W_h @ qv^T  (directly transposed) ----------------------
            ph0 = pg.tile([128, 4 * B], f32, tag="psg")
            for mc in range(4):
                for kc in range(4):
                    nc.tensor.matmul(
                        ph0[:, mc * 64 : (mc + 1) * 64],
                        lhsT=wht[:, kc * H + mc * 128 : kc * H + mc * 128 + 128],
                        rhs=qvt[:, kc * 64 : (kc + 1) * 64],
                        start=(kc == 0),
                        stop=(kc == 3),
                    )
            h0T = const.tile([128, 4 * B], bf16)
            nc.vector.tensor_copy(h0T[:], ph0[:])

            # ---- c0 in tiled layout [128=(hh,b), 256] --------------------------
            pc0 = pg.tile([128, 256], f32, tag="psg")
            for hh in range(2):
                for kc in range(4):
                    nc.tensor.matmul(
                        pc0[64 * hh : 64 * hh + 64, :],
                        lhsT=qvt[:, kc * 64 : (kc + 1) * 64],
                        rhs=wct[:, kc * H + 256 * hh : kc * H + 256 * hh + 256],
                        start=(kc == 0),
                        stop=(kc == 3),
                        tile_position=(0, 64 * hh),
                    )
            c_t = const.tile([128, 256], f32)
            nc.vector.tensor_copy(c_t[:], pc0[:])

            vocab_psum = {}

            def emit_vocab_mms(m, vls, after=None):
                for vl in vls:
                    pvt = vocab_psum[m][vl]
                    for kc in range(4):
                        mm = nc.tensor.matmul(
                            pvt[:],
                            lhsT=H_allT[:, kc * TB + 128 * m : kc * TB + 128 * m + 128],
                            rhs=wvt[:, kc * VS + vl * VT : kc * VS + vl * VT + VT],
                            start=(kc == 0),
                            stop=(kc == 3),
                        )
                        if after is not None and kc == 0:
                            tile.add_dep_helper(
                                mm.ins, after.ins, reason="spread vocab into act window"
                            )

            def emit_vocab_out(m):
                # bias-add fused into the PSUM->SBUF staging (b_vocab is
                # replicated across partitions in bvoc)
                st = stpool.tile([128, VS], bf16, tag="st")
                for vl in range(NVT):
                    nc.vector.tensor_add(
                        st[:, vl * VT : (vl + 1) * VT],
                        vocab_psum[m][vl][:],
                        bvoc[:, vl * VT : (vl + 1) * VT],
                    )
                nc.sync.dma_start(out_d[128 * m : 128 * m + 128, :], st[:])
                del vocab_psum[m]

            # ---- the 64 LSTM steps ---------------------------------------------
            for t in range(T):
                psg0 = pg.tile([128, 512], f32, tag="psg")
                psg1 = pg.tile([128, 512], f32, tag="psg")

                def lhs_h(kc, t=t):
                    if t == 0:
                        return h0T[:, kc * 64 : (kc + 1) * 64]
                    c0 = kc * TB + 64 * (t - 1)
                    return H_allT[:, c0 : c0 + 64]

                # interleave the two column-tile chains (hh=0 on cols 0-63,
                # hh=1 on cols 64-127) so adjacent matmuls hit different col
                # groups and overlap in the PE array. bank0 ([i|g]) first:
                # its activations and i*g run while bank1 ([f|o]) is still in
                # the array, leaving only sigmoid(f) -> f*c -> c -> tanh -> h
                # after the last matmul. h^T chunks are consumed in (0,2,1,3)
                # order so the first matmuls only wait on the first H copy.
                # bias (kc=6) and the x-projection chunks (kc=4,5) lead each
                # chain: they have no dependency on h, so the PE can run them
                # during the previous step's activation tail; only the four
                # h^T chunks trail, so the bank completes ~3 pairs earlier
                for bank, psg in ((0, psg0), (1, psg1)):
                    for kc in (6, 4, 5, 0, 2, 1, 3):
                        for hh in range(2):
                            n0 = bank * 1024 + hh * 512
                            out_ap = psg[64 * hh : 64 * hh + 64, :]
                            tp = (0, 64 * hh)
                            if kc < 4:
                                lhsT = lhs_h(kc)
                                rhs = wcat[:, kc * 2048 + n0 : kc * 2048 + n0 + 512]
                            elif kc < 6:
                                xc = kc - 4
                                lhsT = xst[:, xc * TB + 64 * t : xc * TB + 64 * t + 64]
                                rhs = wcat[:, kc * 2048 + n0 : kc * 2048 + n0 + 512]
                            else:
                                lhsT = ones[0:1, 0:64]
                                rhs = wbias[0:1, n0 : n0 + 512]
                            nc.tensor.matmul(
                                out_ap,
                                lhsT=lhsT,
                                rhs=rhs,
                                start=(kc == 6),
                                stop=(kc == 3),
                                tile_position=tp,
                                skip_group_check=True,
                            )

                # activations: bank0 = [i|g] (lands first), bank1 = [f|o]
                s_ig = work.tile([128, 512], f32, tag="s_ig")
                s_fo = work.tile([128, 512], f32, tag="s_fo")
                igt = work.tile([128, 256], f32, tag="igt")
                fct = work.tile([128, 256], f32, tag="fct")
                tct = work.tile([128, 256], f32, tag="tct")
                h_bf = hpool.tile([128, 256], bf16, tag="h")

                nc.scalar.activation(s_ig[:, 0:256], psg0[:, 0:256], AF.Sigmoid)
                nc.scalar.activation(s_ig[:, 256:512], psg0[:, 256:512], AF.Tanh)
                nc.vector.tensor_mul(igt[:], s_ig[:, 0:256], s_ig[:, 256:512])
                sigf_inst = nc.scalar.activation(
                    s_fo[:, 0:256], psg1[:, 0:256], AF.Sigmoid
                )
                nc.scalar.activation(s_fo[:, 256:512], psg1[:, 256:512], AF.Sigmoid)
                nc.vector.tensor_mul(fct[:], s_fo[:, 0:256], c_t[:])
                nc.vector.tensor_add(c_t[:], fct[:], igt[:])
                nc.scalar.activation(tct[:], c_t[:], AF.Tanh)

                # lagged vocab matmuls fill the PE gap during this step's
                # activation chain; the last tile is pinned behind sigmoid(f)
                # so PE activity spreads across the whole chain window and
                # the HAM clock gate never sees a long idle stretch
                if t >= 2:
                    m = t // 2 - 1
                    if t % 2 == 0:
                        vocab_psum[m] = [
                            pv.tile([128, VT], f32, tag="psv", name=f"psv{m}_{_vl}")
                            for _vl in range(NVT)
                        ]
                        emit_vocab_mms(m, (0,))
                        emit_vocab_mms(m, (1,), after=sigf_inst)
                    else:
                        emit_vocab_mms(m, (2, 3))
                        emit_vocab_mms(m, (4,), after=sigf_inst)

                # h and its transpose, pipelined in unit-halves so the PE
                # starts transposing half 0 while DVE computes half 1
                pst = pt.tile([128, 256], f32, tag="pst")
                for ui in range(2):
                    nc.vector.tensor_mul(
                        h_bf[:, ui * 128 : (ui + 1) * 128],
                        s_fo[:, 256 + ui * 128 : 256 + (ui + 1) * 128],
                        tct[:, ui * 128 : (ui + 1) * 128],
                    )
                    nc.tensor.matmul(
                        pst[:, ui * 128 : (ui + 1) * 128],
                        lhsT=h_bf[:, ui * 128 : (ui + 1) * 128],
                        rhs=ident[:],
                        start=True,
                        stop=True,
                    )
                H_v = H_allT[:].rearrange("p (c n) -> p c n", c=4)
                nc.vector.tensor_copy(
                    H_v[:, 0:3:2, 64 * t : 64 * t + 64],
                    pst[:, 0:128].rearrange("p (c n) -> p c n", c=2),
                )
                nc.scalar.copy(
                    H_v[:, 1:4:2, 64 * t : 64 * t + 64],
                    pst[:, 128:256].rearrange("p (c n) -> p c n", c=2),
                )

                # stage + store the lagged vocab pair
                if t >= 3 and t % 2 == 1:
                    emit_vocab_out(t // 2 - 1)

            # tail: last vocab pair
            m = TB // 128 - 1
            vocab_psum[m] = [pv.tile([128, VT], f32, tag="psv", name=f"psv{m}_{_vl}") for _vl in range(NVT)]
            emit_vocab_mms(m, (0, 1, 2, 3, 4))
            emit_vocab_out(m)


def _host_prep(inputs):
    import ml_dtypes

    bf = ml_dtypes.bfloat16
    f32 = np.float32

    qv = inputs["question_vectors"].astype(f32)
    emb = inputs["emb_table"].astype(f32)
    W_h, W_c = inputs["W_h"].astype(f32), inputs["W_c"].astype(f32)
    W_ih, W_hh = inputs["W_ih"].astype(f32), inputs["W_hh"].astype(f32)
    b_ih, b_hh = inputs["b_ih"].astype(f32), inputs["b_hh"].astype(f32)
    W_vocab, b_vocab = inputs["W_vocab"].astype(f32), inputs["b_vocab"].astype(f32)
    answers = inputs["answers"]

    perm = _gate_perm()
    wcat = np.concatenate(
        [W_hh.T, W_ih.T, (b_ih + b_hh)[None, :]], axis=0
    )[:, perm].astype(bf)                                   # [769, 2048]

    # teacher-forced inputs, gathered on host: [T, B, E] -> x^T [E, T*B]
    xs = np.concatenate(
        [
            np.broadcast_to(emb[START_IDX], (1, B, E)),
            emb[answers[:, :-1]].transpose(1, 0, 2),
        ],
        axis=0,
    )
    xst = np.ascontiguousarray(xs.reshape(TB, E).T).astype(bf)  # [E, TB]

    qvt = np.ascontiguousarray(qv.T).astype(bf)
    wht = np.ascontiguousarray(W_h.T).astype(bf)
    wct = np.ascontiguousarray(W_c.T).astype(bf)
    ident = np.eye(128, dtype=bf)
    ones = np.ones((1, 128), dtype=bf)

    common = dict(
        xst=xst, wcat=wcat, qvt=qvt, wht=wht, wct=wct, ident=ident, ones=ones
    )
    in_maps = []
    for k in range(NCORES):
        wvt = np.concatenate(
            [W_vocab[k * VS : (k + 1) * VS].T, b_vocab[None, k * VS : (k + 1) * VS]],
            axis=0,
        ).astype(bf)                                        # [513, 2500]
        in_maps.append(dict(common, wvt=wvt))
    return in_maps


def _install_ntff_hook():
    """Shim antenv.axon_hooks (absent in this image) so BASS_TRACE=1 works."""
    if "antenv.axon_hooks" in sys.modules:
        return
    try:
        mod = types.ModuleType("antenv.axon_hooks")
        mod._hook = None
        mod.set_axon_ntff_profile_hook = lambda h: setattr(mod, "_hook", h)
        mod.get_axon_ntff_profile_hook = lambda: mod._hook
        sys.modules["antenv.axon_hooks"] = mod
        from trn_agent_boot.trn_boot import _ntff_profile_via_ctypes

        mod.set_axon_ntff_profile_hook(
            _ntff_profile_via_ctypes("/opt/axon/libaxon_pjrt.so")
        )
    except Exception:
        sys.modules.pop("antenv.axon_hooks", None)


def kernel(**inputs):
    inputs = {k: np.asarray(v) for k, v in inputs.items()}
    if os.environ.get("BASS_TRACE"):
        _install_ntff_hook()

    in_maps = _host_prep(inputs)

    nc = bacc.Bacc("TRN2", target_bir_lowering=False, debug=False, num_devices=NCORES)
    build(nc)
    nc.compile()

    res = run_bass_kernel_spmd(nc, in_maps, core_ids=list(range(NCORES)))
    kernel._last_result = res

    outs = [
        res.results[k]["out"].astype(np.float32).reshape(T, B, VS).transpose(1, 0, 2)
        for k in range(NCORES)
    ]
    return np.concatenate(outs, axis=2)

